# revision 26
# baseline (speedup 1.0000x reference)
"""AttentionBlock (GroupNorm + 4-head self-attention + proj + residual) on 8 trn2 cores.

Input  x: (16, 512, 32, 32) fp32.  Data-parallel: 2 images per NeuronCore.

Per-core dataflow (per image, C=512, N=H*W=1024, nh=4, hd=128):
  x (c,n)  --GN stats (DVE reduce + ACT square-accum + tiny PE selector mms)-->
  xn = x*a + b  (one DVE tensor_scalar per 128-chan tile)
  q,k: psum[d,n] = wqkvT[c,d].T @ xn[c,n]      (d on partitions, head-major)
  vT:  psum[n,dv] = xn[c,n].T @ wvT[c,dv]      (v born transposed; no PE transposes)
  per head h:
    scT[j,i] = k_h[c,j].T @ q_h[c,i]           (j on partitions)
    expT = exp(scale*scT)                       (ACT, PSUM->SBUF)
    S[i] += ones.T @ expT                       (PE, accumulated over j-tiles)
    av[c,i] += vT_h[j,c].T @ expT[j,i]          (PE, accumulated over j-tiles)
    r = 1/S  (DVE) -> DRAM -> DMA-broadcast to 128 partitions
    outn[c,i] = av * r                          (DVE, psum x sbuf)
  proj: pp[d,n] = wpT[c,d].T @ outn[c,n]; y = (pp + b_proj) + x   (fused DVE)

All big matmuls run in float32r (1 cycle/row on the PE vs 4 for float32).
"""

import numpy as np

import concourse.bass as bass
import concourse.bacc as bacc
import concourse.tile as tile
from concourse import mybir
from concourse.bass_utils import run_bass_kernel_spmd

F32 = mybir.dt.float32
F32R = mybir.dt.float32r
BF16 = mybir.dt.bfloat16
ATTN_BF16 = False     # q/k/vT/expT in bf16 (scores still accumulate in fp32 PSUM)
AF = mybir.ActivationFunctionType
ALU = mybir.AluOpType
AX = mybir.AxisListType

B, C, H, W = 16, 512, 32, 32
N = H * W                 # 1024
NH, HD = 4, 128
G, GS = 8, 64             # groups, channels per group
NCORES = 8
BPC = B // NCORES         # images per core
CT = C // 128             # 4 channel tiles
NT = N // 128             # 8 spatial tiles
EPS = 1e-5
SCALE = float(HD) ** -0.5
INV_GROUP = 1.0 / (GS * N)   # 1/65536


def r32(ap):
    return ap.bitcast(F32R)


def _emit(ctx, tc, aps):
    nc = tc.nc
    x_d, wqkvT_d, wpT_d, gamma_d, beta_d, bproj_d, y_d = aps[:7]

    consts = ctx.enter_context(tc.tile_pool(name="consts", bufs=1))
    xp = ctx.enter_context(tc.tile_pool(name="xp", bufs=2))
    work = ctx.enter_context(tc.tile_pool(name="work", bufs=1))
    small = ctx.enter_context(tc.tile_pool(name="small", bufs=4))
    expp = ctx.enter_context(tc.tile_pool(name="expp", bufs=3))
    rbcp = ctx.enter_context(tc.tile_pool(name="rbcp", bufs=2))
    dramp = ctx.enter_context(tc.tile_pool(name="dramp", bufs=4, space="DRAM"))
    ps_sc = ctx.enter_context(tc.tile_pool(name="ps_sc", bufs=3, space="PSUM"))
    ps_av = ctx.enter_context(tc.tile_pool(name="ps_av", bufs=3, space="PSUM"))
    ps_s = ctx.enter_context(tc.tile_pool(name="ps_s", bufs=2, space="PSUM"))

    # ---- tiny constants first (sel gates the GN matmuls), then image 0's x,
    # then the big weight DMAs (queues are FIFO: a small transfer queued after
    # a 3MB one waits for all of it)
    eps2 = consts.tile([2, 1], F32)
    nc.vector.memset(eps2, EPS)
    sel = consts.tile([128, 2], F32)
    nc.sync.dma_start(out=sel, in_=aps[7])
    selT2 = consts.tile([2, 128], F32)
    nc.sync.dma_start(out=selT2, in_=aps[8])
    gamma = consts.tile([128, CT], F32)
    nc.sync.dma_start(out=gamma, in_=gamma_d.rearrange("(t p) -> p t", p=128))
    beta = consts.tile([128, CT], F32)
    nc.sync.dma_start(out=beta, in_=beta_d.rearrange("(t p) -> p t", p=128))
    if ATTN_BF16:
        ones = consts.tile([128, 1], BF16)
        nc.vector.memset(ones, 1.0)
    else:
        ones = consts.tile([128, 1], F32R)
        nc.sync.dma_start(out=ones, in_=r32(aps[9]))
    ones32 = consts.tile([128, 1], F32)
    nc.vector.memset(ones32, 1.0)

    x_tiles = []
    x0 = xp.tile([128, CT, N], F32, tag="x", name="x0")
    x0_r = x_d[0].rearrange("(t p) n -> p t n", p=128)
    for ct in range(CT):
        nc.sync.dma_start(out=x0[:, ct, :], in_=x0_r[:, ct, :])
    x_tiles.append(x0)

    wqkvT = consts.tile([128, CT, 3 * C], F32R)
    nc.sync.dma_start(out=wqkvT,
                      in_=r32(wqkvT_d.rearrange("(t p) d -> p t d", p=128)))
    x1 = xp.tile([128, CT, N], F32, tag="x", name="x1")
    x1_r = x_d[1].rearrange("(t p) n -> p t n", p=128)
    for ct in range(CT):
        nc.sync.dma_start(out=x1[:, ct, :], in_=x1_r[:, ct, :])
    x_tiles.append(x1)
    wpT = consts.tile([128, CT, C], F32R)
    nc.sync.dma_start(out=wpT, in_=r32(wpT_d.rearrange("(t p) d -> p t d", p=128)))
    bproj = consts.tile([128, CT], F32)
    nc.sync.dma_start(out=bproj, in_=bproj_d.rearrange("(t p) -> p t", p=128))

    xn_l, q_l, k_l, vT_l, outn_l = {}, {}, {}, {}, {}

    def phase_gn(b):
        x_t = x_tiles[b]
        # ---- group norm --------------------------------------------------
        xn_t = work.tile([128, CT, N], F32R, tag="xn", name=f"xn{b}")
        ss = small.tile([128, CT, 2], F32, tag="ss")
        for ct in range(CT):
            nc.vector.reduce_sum(out=ss[:, ct, 0:1], in_=x_t[:, ct, :], axis=AX.X)
            # squares go to xn_t as scratch (overwritten below)
            nc.scalar.activation(out=xn_t[:, ct, :], in_=x_t[:, ct, :],
                                 func=AF.Square, accum_out=ss[:, ct, 1:2])
        gn_ps = ps_sc.tile([2, CT, 2], F32, tag="sc")
        for ct in range(CT):
            nc.tensor.matmul(gn_ps[:, ct, :], lhsT=sel, rhs=ss[:, ct, :],
                             start=(ct == 0), stop=(ct == CT - 1))
        msq = small.tile([2, CT, 1], F32, tag="msq")
        nc.scalar.activation(out=msq, in_=gn_ps[:, :, 0:1], func=AF.Square)
        var = small.tile([2, CT, 1], F32, tag="var")
        nc.vector.tensor_sub(out=var, in0=gn_ps[:, :, 1:2], in1=msq)
        std = small.tile([2, CT, 1], F32, tag="std")
        nc.scalar.activation(out=std, in_=var, func=AF.Sqrt, bias=eps2)
        mr = small.tile([2, CT, 2], F32, tag="mr")      # [mean, rstd]
        nc.vector.reciprocal(out=mr[:, :, 1:2], in_=std)
        nc.vector.tensor_copy(out=mr[:, :, 0:1], in_=gn_ps[:, :, 0:1])
        for ct in range(CT):
            bc = ps_sc.tile([128, 2], F32, tag="sc")
            nc.tensor.matmul(bc, lhsT=selT2, rhs=mr[:, ct, :],
                             start=True, stop=True)     # [mean_c, rstd_c]
            a_sc = small.tile([128, 1], F32, tag="a_sc")
            nc.vector.tensor_mul(out=a_sc, in0=bc[:, 1:2], in1=gamma[:, ct:ct + 1])
            nb_sc = small.tile([128, 1], F32, tag="nb_sc")  # mean*a - beta
            nc.vector.scalar_tensor_tensor(out=nb_sc, in0=bc[:, 0:1], scalar=a_sc,
                                           in1=beta[:, ct:ct + 1],
                                           op0=ALU.mult, op1=ALU.subtract)
            nc.vector.tensor_scalar(out=xn_t[:, ct, :], in0=x_t[:, ct, :],
                                    scalar1=a_sc, scalar2=nb_sc,
                                    op0=ALU.mult, op1=ALU.subtract)

        xn_l[b] = xn_t

    def phase_qkv(b):
        xn_t = xn_l[b]
        # ---- q, k --------------------------------------------------------
        adt = BF16 if ATTN_BF16 else F32R
        q_t = work.tile([128, NH, N], adt, tag="q", name=f"q{b}", bufs=1)
        k_t = work.tile([128, NH, N], adt, tag="k", name=f"k{b}", bufs=1)
        for which, dst in ((0, q_t), (1, k_t)):
            for h in range(NH):
                dlo = which * C + h * 128
                for nh_ in range(2):
                    nsl = slice(nh_ * 512, (nh_ + 1) * 512)
                    qk_ps = ps_sc.tile([128, 512], F32, tag="sc")
                    for ct in range(CT):
                        nc.tensor.matmul(
                            qk_ps, lhsT=wqkvT[:, ct, dlo:dlo + 128],
                            rhs=xn_t[:, ct, nsl],
                            start=(ct == 0), stop=(ct == CT - 1))
                    if which == 0:
                        nc.scalar.copy(out=dst[:, h, nsl], in_=qk_ps)
                    else:
                        nc.vector.tensor_copy(out=dst[:, h, nsl], in_=qk_ps)

        q_l[b], k_l[b] = q_t, k_t

    def phase_vt(b):
        xn_t = xn_l[b]
        adt = BF16 if ATTN_BF16 else F32R
        # ---- vT ----------------------------------------------------------
        vT_t = work.tile([128, NT, C], adt, tag="vT", name=f"vT{b}", bufs=2)
        for nt in range(NT):
            vt_ps = ps_sc.tile([128, C], F32, tag="sc")
            for ct in range(CT):
                nc.tensor.matmul(vt_ps,
                                 lhsT=xn_t[:, ct, nt * 128:(nt + 1) * 128],
                                 rhs=wqkvT[:, ct, 2 * C:3 * C],
                                 start=(ct == 0), stop=(ct == CT - 1))
            nc.vector.tensor_copy(out=vT_t[:, nt, :], in_=vt_ps)
        vT_l[b] = vT_t

    def phase_b(b):
        q_t, k_t, vT_t = q_l[b], k_l[b], vT_l[b]
        adt = BF16 if ATTN_BF16 else F32R
        outn_t = work.tile([128, NH, N], F32R, tag="outn", name=f"outn{b}", bufs=1)
        for h in range(NH):
            for ih in range(2):
                isl = slice(ih * 512, (ih + 1) * 512)
                s_ps = ps_s.tile([1, 512], F32, tag="s")
                av = ps_av.tile([128, 512], F32, tag="av")
                for jt in range(NT):
                    sc = ps_sc.tile([128, 512], F32, tag="sc")
                    nc.tensor.matmul(sc,
                                     lhsT=k_t[:, h, jt * 128:(jt + 1) * 128],
                                     rhs=q_t[:, h, isl], start=True, stop=True)
                    expT = expp.tile([128, 512], adt, tag="expT")
                    nc.scalar.activation(out=expT, in_=sc, func=AF.Exp, scale=SCALE)
                    nc.tensor.matmul(s_ps, lhsT=ones, rhs=expT,
                                     start=(jt == 0), stop=(jt == NT - 1))
                    nc.tensor.matmul(av,
                                     lhsT=vT_t[:, jt, h * 128:(h + 1) * 128],
                                     rhs=expT,
                                     start=(jt == 0), stop=(jt == NT - 1))
                # 1/S on a (128,4) layout (a (1,512) reciprocal is ~3us: the
                # iterative divide runs on one lane); round-trip through DRAM
                s_sb = small.tile([1, 512], F32, tag="s_sb")
                nc.vector.tensor_copy(out=s_sb, in_=s_ps)
                s128 = small.tile([128, 4], F32, tag="s128")
                nc.sync.dma_start(
                    out=s128,
                    in_=bass.AP(tensor=s_sb.tensor, offset=s_sb.offset,
                                ap=[list(s_sb.ap[0]), [4, 128], [1, 4]]))
                r128 = small.tile([128, 4], F32, tag="r128")
                nc.vector.reciprocal(out=r128, in_=s128)
                r_dram = dramp.tile([512], F32, tag="r_dram")
                nc.sync.dma_start(out=r_dram.rearrange("(p f) -> p f", p=128),
                                  in_=r128)
                r_bc = rbcp.tile([128, 512], F32, tag="r_bc")
                nc.sync.dma_start(
                    out=r_bc,
                    in_=bass.AP(tensor=r_dram.tensor, offset=r_dram.offset,
                                ap=[[0, 128]] + list(r_dram.ap)))
                nc.vector.tensor_mul(out=outn_t[:, h, isl], in0=av, in1=r_bc)
        outn_l[b] = outn_t

    def phase_c(b):
        x_t, outn_t = x_tiles[b], outn_l[b]
        y_t = work.tile([128, CT, N], F32, tag="y", name=f"y{b}")
        for dt in range(CT):
            for nh_ in range(2):
                nsl = slice(nh_ * 512, (nh_ + 1) * 512)
                pp = ps_sc.tile([128, 512], F32, tag="sc")
                for ct in range(CT):
                    nc.tensor.matmul(
                        pp, lhsT=wpT[:, ct, dt * 128:(dt + 1) * 128],
                        rhs=outn_t[:, ct, nsl],
                        start=(ct == 0), stop=(ct == CT - 1))
                nc.vector.scalar_tensor_tensor(out=y_t[:, dt, nsl], in0=pp,
                                               scalar=bproj[:, dt:dt + 1],
                                               in1=x_t[:, dt, nsl],
                                               op0=ALU.add, op1=ALU.add)
        nc.sync.dma_start(out=y_d[b].rearrange("(t p) n -> p t n", p=128), in_=y_t)

    phase_gn(0)
    phase_qkv(0)
    phase_vt(0)
    phase_b(0)
    phase_gn(1)
    phase_vt(1)
    phase_c(0)
    phase_qkv(1)
    phase_b(1)
    phase_c(1)


def _sel_np():
    s = np.zeros((128, 2), dtype=np.float32)
    s[0:GS, 0] = INV_GROUP
    s[GS:128, 1] = INV_GROUP
    return s


def _selT2_np():
    s = np.zeros((2, 128), dtype=np.float32)
    s[0, 0:GS] = 1.0
    s[1, GS:128] = 1.0
    return s


_CACHE = {}


def _build():
    if "nc" in _CACHE:
        return _CACHE["nc"]
    nc = bacc.Bacc("TRN2", target_bir_lowering=False, debug=False)
    x_d = nc.dram_tensor("x", [BPC, C, N], F32, kind="ExternalInput").ap()
    wqkvT_d = nc.dram_tensor("wqkvT", [C, 3 * C], F32, kind="ExternalInput").ap()
    wpT_d = nc.dram_tensor("wpT", [C, C], F32, kind="ExternalInput").ap()
    gamma_d = nc.dram_tensor("gamma", [C], F32, kind="ExternalInput").ap()
    beta_d = nc.dram_tensor("beta", [C], F32, kind="ExternalInput").ap()
    bproj_d = nc.dram_tensor("bproj", [C], F32, kind="ExternalInput").ap()
    sel_d = nc.dram_tensor("sel", [128, 2], F32, kind="ExternalInput").ap()
    selT2_d = nc.dram_tensor("selT2", [2, 128], F32, kind="ExternalInput").ap()
    ones_d = nc.dram_tensor("ones", [128, 1], F32, kind="ExternalInput").ap()
    y_d = nc.dram_tensor("y", [BPC, C, N], F32, kind="ExternalOutput").ap()
    from contextlib import ExitStack
    with tile.TileContext(nc) as tc, ExitStack() as ctx:
        _emit(ctx, tc, (x_d, wqkvT_d, wpT_d, gamma_d, beta_d, bproj_d, y_d,
                        sel_d, selT2_d, ones_d))
    nc.compile()
    _CACHE["nc"] = nc
    return nc


def run(inputs: dict, trace: bool = False, **kw):
    nc = _build()
    x = np.ascontiguousarray(inputs["x"].reshape(B, C, N), dtype=np.float32)
    shared = {
        "wqkvT": np.ascontiguousarray(inputs["w_qkv"].T, dtype=np.float32),
        "wpT": np.ascontiguousarray(inputs["w_proj"].T, dtype=np.float32),
        "gamma": np.ascontiguousarray(inputs["gn_gamma"], dtype=np.float32),
        "beta": np.ascontiguousarray(inputs["gn_beta"], dtype=np.float32),
        "bproj": np.ascontiguousarray(inputs["b_proj"], dtype=np.float32),
        "sel": _sel_np(),
        "selT2": _selT2_np(),
        "ones": np.ones((128, 1), dtype=np.float32),
    }
    in_maps = [dict(shared, x=x[i * BPC:(i + 1) * BPC]) for i in range(NCORES)]
    res = run_bass_kernel_spmd(nc, in_maps, list(range(NCORES)), trace=trace, **kw)
    out = np.concatenate([res.results[i]["y"] for i in range(NCORES)], axis=0)
    return out.reshape(B, C, H, W), res


def kernel(**inputs) -> np.ndarray:
    out, _ = run(inputs)
    return out



# revision 27
# speedup vs baseline: 1.0299x; 1.0299x over previous
"""AttentionBlock (GroupNorm + 4-head self-attention + proj + residual) on 8 trn2 cores.

Input  x: (16, 512, 32, 32) fp32.  Data-parallel: 2 images per NeuronCore.

Per-core dataflow (per image, C=512, N=H*W=1024, nh=4, hd=128):
  x (c,n)  --GN stats (DVE reduce + ACT square-accum + tiny PE selector mms)-->
  xn = x*a + b  (one DVE tensor_scalar per 128-chan tile)
  q,k: psum[d,n] = wqkvT[c,d].T @ xn[c,n]      (d on partitions, head-major)
  vT:  psum[n,dv] = xn[c,n].T @ wvT[c,dv]      (v born transposed; no PE transposes)
  per head h:
    scT[j,i] = k_h[c,j].T @ q_h[c,i]           (j on partitions)
    expT = exp(scale*scT)                       (ACT, PSUM->SBUF)
    S[i] += ones.T @ expT                       (PE, accumulated over j-tiles)
    av[c,i] += vT_h[j,c].T @ expT[j,i]          (PE, accumulated over j-tiles)
    r = 1/S  (DVE) -> DRAM -> DMA-broadcast to 128 partitions
    outn[c,i] = av * r                          (DVE, psum x sbuf)
  proj: pp[d,n] = wpT[c,d].T @ outn[c,n]; y = (pp + b_proj) + x   (fused DVE)

All big matmuls run in float32r (1 cycle/row on the PE vs 4 for float32).
"""

import numpy as np

import concourse.bass as bass
import concourse.bacc as bacc
import concourse.tile as tile
from concourse import mybir
from concourse.bass_utils import run_bass_kernel_spmd

F32 = mybir.dt.float32
F32R = mybir.dt.float32r
BF16 = mybir.dt.bfloat16
ATTN_BF16 = False     # q/k/vT/expT in bf16 (scores still accumulate in fp32 PSUM)
AF = mybir.ActivationFunctionType
ALU = mybir.AluOpType
AX = mybir.AxisListType

B, C, H, W = 16, 512, 32, 32
N = H * W                 # 1024
NH, HD = 4, 128
G, GS = 8, 64             # groups, channels per group
NCORES = 8
BPC = B // NCORES         # images per core
CT = C // 128             # 4 channel tiles
NT = N // 128             # 8 spatial tiles
EPS = 1e-5
SCALE = float(HD) ** -0.5
INV_GROUP = 1.0 / (GS * N)   # 1/65536


def r32(ap):
    return ap.bitcast(F32R)


def _emit(ctx, tc, aps):
    nc = tc.nc
    x_d, wqkvT_d, wpT_d, gamma_d, beta_d, bproj_d, y_d = aps[:7]

    consts = ctx.enter_context(tc.tile_pool(name="consts", bufs=1))
    xp = ctx.enter_context(tc.tile_pool(name="xp", bufs=2))
    work = ctx.enter_context(tc.tile_pool(name="work", bufs=1))
    small = ctx.enter_context(tc.tile_pool(name="small", bufs=4))
    expp = ctx.enter_context(tc.tile_pool(name="expp", bufs=3))
    rbcp = ctx.enter_context(tc.tile_pool(name="rbcp", bufs=2))
    dramp = ctx.enter_context(tc.tile_pool(name="dramp", bufs=4, space="DRAM"))
    ps_sc = ctx.enter_context(tc.tile_pool(name="ps_sc", bufs=3, space="PSUM"))
    ps_av = ctx.enter_context(tc.tile_pool(name="ps_av", bufs=3, space="PSUM"))
    ps_s = ctx.enter_context(tc.tile_pool(name="ps_s", bufs=2, space="PSUM"))

    # ---- tiny constants first (sel gates the GN matmuls), then image 0's x,
    # then the big weight DMAs (queues are FIFO: a small transfer queued after
    # a 3MB one waits for all of it)
    eps2 = consts.tile([2, 1], F32)
    nc.vector.memset(eps2, EPS)
    sel = consts.tile([128, 2], F32)
    nc.sync.dma_start(out=sel, in_=aps[7])
    selT2 = consts.tile([2, 128], F32)
    nc.sync.dma_start(out=selT2, in_=aps[8])
    gamma = consts.tile([128, CT], F32)
    nc.sync.dma_start(out=gamma, in_=gamma_d.rearrange("(t p) -> p t", p=128))
    beta = consts.tile([128, CT], F32)
    nc.sync.dma_start(out=beta, in_=beta_d.rearrange("(t p) -> p t", p=128))
    if ATTN_BF16:
        ones = consts.tile([128, 1], BF16)
        nc.vector.memset(ones, 1.0)
    else:
        ones = consts.tile([128, 1], F32R)
        nc.sync.dma_start(out=ones, in_=r32(aps[9]))

    x_tiles = []
    x0 = xp.tile([128, CT, N], F32, tag="x", name="x0")
    x0_r = x_d[0].rearrange("(t p) n -> p t n", p=128)
    for ct in range(CT):
        nc.sync.dma_start(out=x0[:, ct, :], in_=x0_r[:, ct, :])
    x_tiles.append(x0)

    wqkvT = consts.tile([128, CT, 3 * C], F32R)
    wq_r = r32(wqkvT_d.rearrange("(t p) d -> p t d", p=128))
    for ct in range(CT):
        nc.sync.dma_start(out=wqkvT[:, ct, :], in_=wq_r[:, ct, :])
    x1 = xp.tile([128, CT, N], F32, tag="x", name="x1")
    x1_r = x_d[1].rearrange("(t p) n -> p t n", p=128)
    for ct in range(CT):
        nc.sync.dma_start(out=x1[:, ct, :], in_=x1_r[:, ct, :])
    x_tiles.append(x1)
    wpT = consts.tile([128, CT, C], F32R)
    nc.sync.dma_start(out=wpT, in_=r32(wpT_d.rearrange("(t p) d -> p t d", p=128)))
    bproj = consts.tile([128, CT], F32)
    nc.sync.dma_start(out=bproj, in_=bproj_d.rearrange("(t p) -> p t", p=128))

    xn_l, q_l, k_l, vT_l, outn_l = {}, {}, {}, {}, {}

    def phase_gn(b):
        x_t = x_tiles[b]
        # ---- group norm --------------------------------------------------
        xn_t = work.tile([128, CT, N], F32R, tag="xn", name=f"xn{b}")
        ss = small.tile([128, CT, 2], F32, tag="ss")
        for ct in range(CT):
            nc.vector.reduce_sum(out=ss[:, ct, 0:1], in_=x_t[:, ct, :], axis=AX.X)
            # squares go to xn_t as scratch (overwritten below)
            nc.scalar.activation(out=xn_t[:, ct, :], in_=x_t[:, ct, :],
                                 func=AF.Square, accum_out=ss[:, ct, 1:2])
        gn_ps = ps_sc.tile([2, CT, 2], F32, tag="sc")
        for ct in range(CT):
            nc.tensor.matmul(gn_ps[:, ct, :], lhsT=sel, rhs=ss[:, ct, :],
                             start=(ct == 0), stop=(ct == CT - 1))
        msq = small.tile([2, CT, 1], F32, tag="msq")
        nc.scalar.activation(out=msq, in_=gn_ps[:, :, 0:1], func=AF.Square)
        var = small.tile([2, CT, 1], F32, tag="var")
        nc.vector.tensor_sub(out=var, in0=gn_ps[:, :, 1:2], in1=msq)
        std = small.tile([2, CT, 1], F32, tag="std")
        nc.scalar.activation(out=std, in_=var, func=AF.Sqrt, bias=eps2)
        mr = small.tile([2, CT, 2], F32, tag="mr")      # [mean, rstd]
        nc.vector.reciprocal(out=mr[:, :, 1:2], in_=std)
        nc.vector.tensor_copy(out=mr[:, :, 0:1], in_=gn_ps[:, :, 0:1])
        for ct in range(CT):
            bc = ps_sc.tile([128, 2], F32, tag="sc")
            nc.tensor.matmul(bc, lhsT=selT2, rhs=mr[:, ct, :],
                             start=True, stop=True)     # [mean_c, rstd_c]
            a_sc = small.tile([128, 1], F32, tag="a_sc")
            nc.vector.tensor_mul(out=a_sc, in0=bc[:, 1:2], in1=gamma[:, ct:ct + 1])
            nb_sc = small.tile([128, 1], F32, tag="nb_sc")  # mean*a - beta
            nc.vector.scalar_tensor_tensor(out=nb_sc, in0=bc[:, 0:1], scalar=a_sc,
                                           in1=beta[:, ct:ct + 1],
                                           op0=ALU.mult, op1=ALU.subtract)
            nc.vector.tensor_scalar(out=xn_t[:, ct, :], in0=x_t[:, ct, :],
                                    scalar1=a_sc, scalar2=nb_sc,
                                    op0=ALU.mult, op1=ALU.subtract)

        xn_l[b] = xn_t

    def phase_qkv(b):
        xn_t = xn_l[b]
        # ---- q, k --------------------------------------------------------
        adt = BF16 if ATTN_BF16 else F32R
        q_t = work.tile([128, NH, N], adt, tag="q", name=f"q{b}", bufs=1)
        k_t = work.tile([128, NH, N], adt, tag="k", name=f"k{b}", bufs=1)
        for which, dst in ((0, q_t), (1, k_t)):
            for h in range(NH):
                dlo = which * C + h * 128
                for nh_ in range(2):
                    nsl = slice(nh_ * 512, (nh_ + 1) * 512)
                    qk_ps = ps_sc.tile([128, 512], F32, tag="sc")
                    for ct in range(CT):
                        nc.tensor.matmul(
                            qk_ps, lhsT=wqkvT[:, ct, dlo:dlo + 128],
                            rhs=xn_t[:, ct, nsl],
                            start=(ct == 0), stop=(ct == CT - 1))
                    if which == 0:
                        nc.scalar.copy(out=dst[:, h, nsl], in_=qk_ps)
                    else:
                        nc.vector.tensor_copy(out=dst[:, h, nsl], in_=qk_ps)

        q_l[b], k_l[b] = q_t, k_t

    def phase_vt(b):
        xn_t = xn_l[b]
        adt = BF16 if ATTN_BF16 else F32R
        # ---- vT ----------------------------------------------------------
        vT_t = work.tile([128, NT, C], adt, tag="vT", name=f"vT{b}", bufs=2)
        for nt in range(NT):
            vt_ps = ps_sc.tile([128, C], F32, tag="sc")
            for ct in range(CT):
                nc.tensor.matmul(vt_ps,
                                 lhsT=xn_t[:, ct, nt * 128:(nt + 1) * 128],
                                 rhs=wqkvT[:, ct, 2 * C:3 * C],
                                 start=(ct == 0), stop=(ct == CT - 1))
            nc.vector.tensor_copy(out=vT_t[:, nt, :], in_=vt_ps)
        vT_l[b] = vT_t

    def phase_b(b):
        q_t, k_t, vT_t = q_l[b], k_l[b], vT_l[b]
        adt = BF16 if ATTN_BF16 else F32R
        outn_t = work.tile([128, NH, N], F32R, tag="outn", name=f"outn{b}", bufs=1)
        for h in range(NH):
            for ih in range(2):
                isl = slice(ih * 512, (ih + 1) * 512)
                s_ps = ps_s.tile([1, 512], F32, tag="s")
                av = ps_av.tile([128, 512], F32, tag="av")
                for jt in range(NT):
                    sc = ps_sc.tile([128, 512], F32, tag="sc")
                    nc.tensor.matmul(sc,
                                     lhsT=k_t[:, h, jt * 128:(jt + 1) * 128],
                                     rhs=q_t[:, h, isl], start=True, stop=True)
                    expT = expp.tile([128, 512], adt, tag="expT")
                    nc.scalar.activation(out=expT, in_=sc, func=AF.Exp, scale=SCALE)
                    nc.tensor.matmul(s_ps, lhsT=ones, rhs=expT,
                                     start=(jt == 0), stop=(jt == NT - 1))
                    nc.tensor.matmul(av,
                                     lhsT=vT_t[:, jt, h * 128:(h + 1) * 128],
                                     rhs=expT,
                                     start=(jt == 0), stop=(jt == NT - 1))
                # 1/S on a (128,4) layout (a (1,512) reciprocal is ~3us: the
                # iterative divide runs on one lane); round-trip through DRAM
                s_sb = small.tile([1, 512], F32, tag="s_sb")
                nc.vector.tensor_copy(out=s_sb, in_=s_ps)
                s128 = small.tile([128, 4], F32, tag="s128")
                nc.sync.dma_start(
                    out=s128,
                    in_=bass.AP(tensor=s_sb.tensor, offset=s_sb.offset,
                                ap=[list(s_sb.ap[0]), [4, 128], [1, 4]]))
                r128 = small.tile([128, 4], F32, tag="r128")
                nc.vector.reciprocal(out=r128, in_=s128)
                r_dram = dramp.tile([512], F32, tag="r_dram")
                nc.sync.dma_start(out=r_dram.rearrange("(p f) -> p f", p=128),
                                  in_=r128)
                r_bc = rbcp.tile([128, 512], F32, tag="r_bc")
                nc.sync.dma_start(
                    out=r_bc,
                    in_=bass.AP(tensor=r_dram.tensor, offset=r_dram.offset,
                                ap=[[0, 128]] + list(r_dram.ap)))
                nc.vector.tensor_mul(out=outn_t[:, h, isl], in0=av, in1=r_bc)
        outn_l[b] = outn_t

    def phase_c(b):
        x_t, outn_t = x_tiles[b], outn_l[b]
        y_t = work.tile([128, CT, N], F32, tag="y", name=f"y{b}")
        for dt in range(CT):
            for nh_ in range(2):
                nsl = slice(nh_ * 512, (nh_ + 1) * 512)
                pp = ps_sc.tile([128, 512], F32, tag="sc")
                for ct in range(CT):
                    nc.tensor.matmul(
                        pp, lhsT=wpT[:, ct, dt * 128:(dt + 1) * 128],
                        rhs=outn_t[:, ct, nsl],
                        start=(ct == 0), stop=(ct == CT - 1))
                nc.vector.scalar_tensor_tensor(out=y_t[:, dt, nsl], in0=pp,
                                               scalar=bproj[:, dt:dt + 1],
                                               in1=x_t[:, dt, nsl],
                                               op0=ALU.add, op1=ALU.add)
            nc.sync.dma_start(
                out=y_d[b].rearrange("(t p) n -> p t n", p=128)[:, dt, :],
                in_=y_t[:, dt, :])

    phase_gn(0)
    phase_qkv(0)
    phase_vt(0)
    phase_b(0)
    phase_gn(1)
    phase_vt(1)
    phase_c(0)
    phase_qkv(1)
    phase_b(1)
    phase_c(1)


def _sel_np():
    s = np.zeros((128, 2), dtype=np.float32)
    s[0:GS, 0] = INV_GROUP
    s[GS:128, 1] = INV_GROUP
    return s


def _selT2_np():
    s = np.zeros((2, 128), dtype=np.float32)
    s[0, 0:GS] = 1.0
    s[1, GS:128] = 1.0
    return s


_CACHE = {}


def _build():
    if "nc" in _CACHE:
        return _CACHE["nc"]
    nc = bacc.Bacc("TRN2", target_bir_lowering=False, debug=False)
    x_d = nc.dram_tensor("x", [BPC, C, N], F32, kind="ExternalInput").ap()
    wqkvT_d = nc.dram_tensor("wqkvT", [C, 3 * C], F32, kind="ExternalInput").ap()
    wpT_d = nc.dram_tensor("wpT", [C, C], F32, kind="ExternalInput").ap()
    gamma_d = nc.dram_tensor("gamma", [C], F32, kind="ExternalInput").ap()
    beta_d = nc.dram_tensor("beta", [C], F32, kind="ExternalInput").ap()
    bproj_d = nc.dram_tensor("bproj", [C], F32, kind="ExternalInput").ap()
    sel_d = nc.dram_tensor("sel", [128, 2], F32, kind="ExternalInput").ap()
    selT2_d = nc.dram_tensor("selT2", [2, 128], F32, kind="ExternalInput").ap()
    ones_d = nc.dram_tensor("ones", [128, 1], F32, kind="ExternalInput").ap()
    y_d = nc.dram_tensor("y", [BPC, C, N], F32, kind="ExternalOutput").ap()
    from contextlib import ExitStack
    with tile.TileContext(nc) as tc, ExitStack() as ctx:
        _emit(ctx, tc, (x_d, wqkvT_d, wpT_d, gamma_d, beta_d, bproj_d, y_d,
                        sel_d, selT2_d, ones_d))
    nc.compile()
    _CACHE["nc"] = nc
    return nc


def run(inputs: dict, trace: bool = False, **kw):
    nc = _build()
    x = np.ascontiguousarray(inputs["x"].reshape(B, C, N), dtype=np.float32)
    shared = {
        "wqkvT": np.ascontiguousarray(inputs["w_qkv"].T, dtype=np.float32),
        "wpT": np.ascontiguousarray(inputs["w_proj"].T, dtype=np.float32),
        "gamma": np.ascontiguousarray(inputs["gn_gamma"], dtype=np.float32),
        "beta": np.ascontiguousarray(inputs["gn_beta"], dtype=np.float32),
        "bproj": np.ascontiguousarray(inputs["b_proj"], dtype=np.float32),
        "sel": _sel_np(),
        "selT2": _selT2_np(),
        "ones": np.ones((128, 1), dtype=np.float32),
    }
    in_maps = [dict(shared, x=x[i * BPC:(i + 1) * BPC]) for i in range(NCORES)]
    res = run_bass_kernel_spmd(nc, in_maps, list(range(NCORES)), trace=trace, **kw)
    out = np.concatenate([res.results[i]["y"] for i in range(NCORES)], axis=0)
    return out.reshape(B, C, H, W), res


def kernel(**inputs) -> np.ndarray:
    out, _ = run(inputs)
    return out



# revision 28
# speedup vs baseline: 1.0906x; 1.0589x over previous
"""AttentionBlock (GroupNorm + 4-head self-attention + proj + residual) on 8 trn2 cores.

Input  x: (16, 512, 32, 32) fp32.  Data-parallel: 2 images per NeuronCore.

Per-core dataflow (per image, C=512, N=H*W=1024, nh=4, hd=128):
  x (c,n)  --GN stats (DVE reduce + ACT square-accum + tiny PE selector mms)-->
  xn = x*a + b  (one DVE tensor_scalar per 128-chan tile)
  q,k: psum[d,n] = wqkvT[c,d].T @ xn[c,n]      (d on partitions, head-major)
  vT:  psum[n,dv] = xn[c,n].T @ wvT[c,dv]      (v born transposed; no PE transposes)
  per head h:
    scT[j,i] = k_h[c,j].T @ q_h[c,i]           (j on partitions)
    expT = exp(scale*scT)                       (ACT, PSUM->SBUF)
    S[i] += ones.T @ expT                       (PE, accumulated over j-tiles)
    av[c,i] += vT_h[j,c].T @ expT[j,i]          (PE, accumulated over j-tiles)
    r = 1/S  (DVE) -> DRAM -> DMA-broadcast to 128 partitions
    outn[c,i] = av * r                          (DVE, psum x sbuf)
  proj: pp[d,n] = wpT[c,d].T @ outn[c,n]; y = (pp + b_proj) + x   (fused DVE)

All big matmuls run in float32r (1 cycle/row on the PE vs 4 for float32).
"""

import numpy as np

import concourse.bass as bass
import concourse.bacc as bacc
import concourse.tile as tile
from concourse import mybir
from concourse.bass_utils import run_bass_kernel_spmd

F32 = mybir.dt.float32
F32R = mybir.dt.float32r
BF16 = mybir.dt.bfloat16
ATTN_BF16 = False     # q/k/vT/expT in bf16 (scores still accumulate in fp32 PSUM)
AF = mybir.ActivationFunctionType
ALU = mybir.AluOpType
AX = mybir.AxisListType

B, C, H, W = 16, 512, 32, 32
N = H * W                 # 1024
NH, HD = 4, 128
G, GS = 8, 64             # groups, channels per group
NCORES = 8
BPC = B // NCORES         # images per core
CT = C // 128             # 4 channel tiles
NT = N // 128             # 8 spatial tiles
EPS = 1e-5
SCALE = float(HD) ** -0.5
INV_GROUP = 1.0 / (GS * N)   # 1/65536


def r32(ap):
    return ap.bitcast(F32R)


def _emit(ctx, tc, aps):
    nc = tc.nc
    x_d, wqkvT_d, wpT_d, gamma_d, beta_d, bproj_d, y_d = aps[:7]

    consts = ctx.enter_context(tc.tile_pool(name="consts", bufs=1))
    xp = ctx.enter_context(tc.tile_pool(name="xp", bufs=2))
    work = ctx.enter_context(tc.tile_pool(name="work", bufs=1))
    small = ctx.enter_context(tc.tile_pool(name="small", bufs=4))
    expp = ctx.enter_context(tc.tile_pool(name="expp", bufs=3))
    rbcp = ctx.enter_context(tc.tile_pool(name="rbcp", bufs=2))
    dramp = ctx.enter_context(tc.tile_pool(name="dramp", bufs=4, space="DRAM"))
    ps_sc = ctx.enter_context(tc.tile_pool(name="ps_sc", bufs=3, space="PSUM"))
    ps_av = ctx.enter_context(tc.tile_pool(name="ps_av", bufs=3, space="PSUM"))
    ps_s = ctx.enter_context(tc.tile_pool(name="ps_s", bufs=2, space="PSUM"))

    # ---- tiny constants first (sel gates the GN matmuls), then image 0's x,
    # then the big weight DMAs (queues are FIFO: a small transfer queued after
    # a 3MB one waits for all of it)
    eps2 = consts.tile([2, 1], F32)
    nc.vector.memset(eps2, EPS)
    sel = consts.tile([128, 2], F32)
    nc.sync.dma_start(out=sel, in_=aps[7])
    selT2 = consts.tile([2, 128], F32)
    nc.sync.dma_start(out=selT2, in_=aps[8])
    gamma = consts.tile([128, CT], F32)
    nc.sync.dma_start(out=gamma, in_=gamma_d.rearrange("(t p) -> p t", p=128))
    beta = consts.tile([128, CT], F32)
    nc.sync.dma_start(out=beta, in_=beta_d.rearrange("(t p) -> p t", p=128))
    if ATTN_BF16:
        ones = consts.tile([128, 1], BF16)
        nc.vector.memset(ones, 1.0)
    else:
        ones = consts.tile([128, 1], F32R)
        nc.sync.dma_start(out=ones, in_=r32(aps[9]))

    x_tiles = []
    x0 = xp.tile([128, CT, N], F32, tag="x", name="x0")
    x0_r = x_d[0].rearrange("(t p) n -> p t n", p=128)
    for ct in range(CT):
        nc.sync.dma_start(out=x0[:, ct, :], in_=x0_r[:, ct, :])
    x_tiles.append(x0)

    wqkvT = consts.tile([128, CT, 3 * C], F32R)
    wq_r = r32(wqkvT_d.rearrange("(t p) d -> p t d", p=128))
    for ct in range(CT):
        nc.sync.dma_start(out=wqkvT[:, ct, :], in_=wq_r[:, ct, :])
    x1 = xp.tile([128, CT, N], F32, tag="x", name="x1")
    x1_r = x_d[1].rearrange("(t p) n -> p t n", p=128)
    for ct in range(CT):
        nc.sync.dma_start(out=x1[:, ct, :], in_=x1_r[:, ct, :])
    x_tiles.append(x1)
    wpT = consts.tile([128, CT, C], F32R)
    nc.sync.dma_start(out=wpT, in_=r32(wpT_d.rearrange("(t p) d -> p t d", p=128)))
    bproj = consts.tile([128, CT], F32)
    nc.sync.dma_start(out=bproj, in_=bproj_d.rearrange("(t p) -> p t", p=128))

    xn_l, q_l, k_l, vT_l, outn_l = {}, {}, {}, {}, {}

    def phase_gn(b):
        x_t = x_tiles[b]
        # ---- group norm --------------------------------------------------
        xn_t = work.tile([128, CT, N], F32R, tag="xn", name=f"xn{b}")
        ss = small.tile([128, CT, 2], F32, tag="ss")
        for ct in range(CT):
            nc.vector.reduce_sum(out=ss[:, ct, 0:1], in_=x_t[:, ct, :], axis=AX.X)
            # squares go to xn_t as scratch (overwritten below)
            nc.scalar.activation(out=xn_t[:, ct, :], in_=x_t[:, ct, :],
                                 func=AF.Square, accum_out=ss[:, ct, 1:2])
        gn_ps = ps_sc.tile([2, CT, 2], F32, tag="sc")
        for ct in range(CT):
            nc.tensor.matmul(gn_ps[:, ct, :], lhsT=sel, rhs=ss[:, ct, :],
                             start=(ct == 0), stop=(ct == CT - 1))
        msq = small.tile([2, CT, 1], F32, tag="msq")
        nc.scalar.activation(out=msq, in_=gn_ps[:, :, 0:1], func=AF.Square)
        var = small.tile([2, CT, 1], F32, tag="var")
        nc.vector.tensor_sub(out=var, in0=gn_ps[:, :, 1:2], in1=msq)
        std = small.tile([2, CT, 1], F32, tag="std")
        nc.scalar.activation(out=std, in_=var, func=AF.Sqrt, bias=eps2)
        mr = small.tile([2, CT, 2], F32, tag="mr")      # [mean, rstd]
        nc.vector.reciprocal(out=mr[:, :, 1:2], in_=std)
        nc.vector.tensor_copy(out=mr[:, :, 0:1], in_=gn_ps[:, :, 0:1])
        for ct in range(CT):
            bc = ps_sc.tile([128, 2], F32, tag="sc")
            nc.tensor.matmul(bc, lhsT=selT2, rhs=mr[:, ct, :],
                             start=True, stop=True)     # [mean_c, rstd_c]
            a_sc = small.tile([128, 1], F32, tag="a_sc")
            nc.vector.tensor_mul(out=a_sc, in0=bc[:, 1:2], in1=gamma[:, ct:ct + 1])
            nb_sc = small.tile([128, 1], F32, tag="nb_sc")  # mean*a - beta
            nc.vector.scalar_tensor_tensor(out=nb_sc, in0=bc[:, 0:1], scalar=a_sc,
                                           in1=beta[:, ct:ct + 1],
                                           op0=ALU.mult, op1=ALU.subtract)
            nc.vector.tensor_scalar(out=xn_t[:, ct, :], in0=x_t[:, ct, :],
                                    scalar1=a_sc, scalar2=nb_sc,
                                    op0=ALU.mult, op1=ALU.subtract)

        xn_l[b] = xn_t

    def phase_qkv(b):
        xn_t = xn_l[b]
        # ---- q, k --------------------------------------------------------
        adt = BF16 if ATTN_BF16 else F32R
        q_t = work.tile([128, NH, N], adt, tag="q", name=f"q{b}", bufs=1)
        k_t = work.tile([128, NH, N], adt, tag="k", name=f"k{b}", bufs=1)
        for which, dst in ((0, q_t), (1, k_t)):
            for h in range(NH):
                dlo = which * C + h * 128
                for nh_ in range(2):
                    nsl = slice(nh_ * 512, (nh_ + 1) * 512)
                    qk_ps = ps_sc.tile([128, 512], F32, tag="sc")
                    for ct in range(CT):
                        nc.tensor.matmul(
                            qk_ps, lhsT=wqkvT[:, ct, dlo:dlo + 128],
                            rhs=xn_t[:, ct, nsl],
                            start=(ct == 0), stop=(ct == CT - 1))
                    if which == 0:
                        nc.scalar.copy(out=dst[:, h, nsl], in_=qk_ps)
                    else:
                        nc.vector.tensor_copy(out=dst[:, h, nsl], in_=qk_ps)

        q_l[b], k_l[b] = q_t, k_t

    def phase_vt(b):
        xn_t = xn_l[b]
        adt = BF16 if ATTN_BF16 else F32R
        # ---- vT ----------------------------------------------------------
        vT_t = work.tile([128, NT, C], adt, tag="vT", name=f"vT{b}", bufs=2)
        for nt in range(NT):
            vt_ps = ps_sc.tile([128, C], F32, tag="sc")
            for ct in range(CT):
                nc.tensor.matmul(vt_ps,
                                 lhsT=xn_t[:, ct, nt * 128:(nt + 1) * 128],
                                 rhs=wqkvT[:, ct, 2 * C:3 * C],
                                 start=(ct == 0), stop=(ct == CT - 1))
            nc.vector.tensor_copy(out=vT_t[:, nt, :], in_=vt_ps)
        vT_l[b] = vT_t

    def phase_b(b):
        q_t, k_t, vT_t = q_l[b], k_l[b], vT_l[b]
        adt = BF16 if ATTN_BF16 else F32R
        # one tile per head: proj's per-head reads then depend only on that
        # head's normalize, not on the whole interleaved outn write pattern
        outn_hs = []
        for h in range(NH):
            outn_h = work.tile([128, N], F32R, tag="outn", name=f"outn{b}_{h}",
                               bufs=4)
            outn_hs.append(outn_h)
            for ih in range(2):
                isl = slice(ih * 512, (ih + 1) * 512)
                s_ps = ps_s.tile([1, 512], F32, tag="s")
                av = ps_av.tile([128, 512], F32, tag="av")
                for jt in range(NT):
                    sc = ps_sc.tile([128, 512], F32, tag="sc")
                    nc.tensor.matmul(sc,
                                     lhsT=k_t[:, h, jt * 128:(jt + 1) * 128],
                                     rhs=q_t[:, h, isl], start=True, stop=True)
                    expT = expp.tile([128, 512], adt, tag="expT")
                    nc.scalar.activation(out=expT, in_=sc, func=AF.Exp, scale=SCALE)
                    nc.tensor.matmul(s_ps, lhsT=ones, rhs=expT,
                                     start=(jt == 0), stop=(jt == NT - 1))
                    nc.tensor.matmul(av,
                                     lhsT=vT_t[:, jt, h * 128:(h + 1) * 128],
                                     rhs=expT,
                                     start=(jt == 0), stop=(jt == NT - 1))
                # 1/S on a (128,4) layout (a (1,512) reciprocal is ~3us: the
                # iterative divide runs on one lane); round-trip through DRAM
                s_sb = small.tile([1, 512], F32, tag="s_sb")
                nc.vector.tensor_copy(out=s_sb, in_=s_ps)
                s128 = small.tile([128, 4], F32, tag="s128")
                nc.sync.dma_start(
                    out=s128,
                    in_=bass.AP(tensor=s_sb.tensor, offset=s_sb.offset,
                                ap=[list(s_sb.ap[0]), [4, 128], [1, 4]]))
                r128 = small.tile([128, 4], F32, tag="r128")
                nc.vector.reciprocal(out=r128, in_=s128)
                r_dram = dramp.tile([512], F32, tag="r_dram")
                nc.sync.dma_start(out=r_dram.rearrange("(p f) -> p f", p=128),
                                  in_=r128)
                r_bc = rbcp.tile([128, 512], F32, tag="r_bc")
                nc.sync.dma_start(
                    out=r_bc,
                    in_=bass.AP(tensor=r_dram.tensor, offset=r_dram.offset,
                                ap=[[0, 128]] + list(r_dram.ap)))
                nc.vector.tensor_mul(out=outn_hs[h][:, isl], in0=av, in1=r_bc)
        outn_l[b] = outn_hs

    def phase_c(b):
        x_t, outn_hs = x_tiles[b], outn_l[b]
        y_t = work.tile([128, CT, N], F32, tag="y", name=f"y{b}")
        for dt in range(CT):
            for nh_ in range(2):
                nsl = slice(nh_ * 512, (nh_ + 1) * 512)
                pp = ps_sc.tile([128, 512], F32, tag="sc")
                for ct in range(CT):
                    nc.tensor.matmul(
                        pp, lhsT=wpT[:, ct, dt * 128:(dt + 1) * 128],
                        rhs=outn_hs[ct][:, nsl],
                        start=(ct == 0), stop=(ct == CT - 1))
                nc.vector.scalar_tensor_tensor(out=y_t[:, dt, nsl], in0=pp,
                                               scalar=bproj[:, dt:dt + 1],
                                               in1=x_t[:, dt, nsl],
                                               op0=ALU.add, op1=ALU.add)
            nc.sync.dma_start(
                out=y_d[b].rearrange("(t p) n -> p t n", p=128)[:, dt, :],
                in_=y_t[:, dt, :])

    phase_gn(0)
    phase_qkv(0)
    phase_vt(0)
    phase_b(0)
    phase_gn(1)
    phase_vt(1)
    phase_c(0)
    phase_qkv(1)
    phase_b(1)
    phase_c(1)


def _sel_np():
    s = np.zeros((128, 2), dtype=np.float32)
    s[0:GS, 0] = INV_GROUP
    s[GS:128, 1] = INV_GROUP
    return s


def _selT2_np():
    s = np.zeros((2, 128), dtype=np.float32)
    s[0, 0:GS] = 1.0
    s[1, GS:128] = 1.0
    return s


_CACHE = {}


def _build():
    if "nc" in _CACHE:
        return _CACHE["nc"]
    nc = bacc.Bacc("TRN2", target_bir_lowering=False, debug=False)
    x_d = nc.dram_tensor("x", [BPC, C, N], F32, kind="ExternalInput").ap()
    wqkvT_d = nc.dram_tensor("wqkvT", [C, 3 * C], F32, kind="ExternalInput").ap()
    wpT_d = nc.dram_tensor("wpT", [C, C], F32, kind="ExternalInput").ap()
    gamma_d = nc.dram_tensor("gamma", [C], F32, kind="ExternalInput").ap()
    beta_d = nc.dram_tensor("beta", [C], F32, kind="ExternalInput").ap()
    bproj_d = nc.dram_tensor("bproj", [C], F32, kind="ExternalInput").ap()
    sel_d = nc.dram_tensor("sel", [128, 2], F32, kind="ExternalInput").ap()
    selT2_d = nc.dram_tensor("selT2", [2, 128], F32, kind="ExternalInput").ap()
    ones_d = nc.dram_tensor("ones", [128, 1], F32, kind="ExternalInput").ap()
    y_d = nc.dram_tensor("y", [BPC, C, N], F32, kind="ExternalOutput").ap()
    from contextlib import ExitStack
    with tile.TileContext(nc) as tc, ExitStack() as ctx:
        _emit(ctx, tc, (x_d, wqkvT_d, wpT_d, gamma_d, beta_d, bproj_d, y_d,
                        sel_d, selT2_d, ones_d))
    nc.compile()
    _CACHE["nc"] = nc
    return nc


def run(inputs: dict, trace: bool = False, **kw):
    nc = _build()
    x = np.ascontiguousarray(inputs["x"].reshape(B, C, N), dtype=np.float32)
    shared = {
        "wqkvT": np.ascontiguousarray(inputs["w_qkv"].T, dtype=np.float32),
        "wpT": np.ascontiguousarray(inputs["w_proj"].T, dtype=np.float32),
        "gamma": np.ascontiguousarray(inputs["gn_gamma"], dtype=np.float32),
        "beta": np.ascontiguousarray(inputs["gn_beta"], dtype=np.float32),
        "bproj": np.ascontiguousarray(inputs["b_proj"], dtype=np.float32),
        "sel": _sel_np(),
        "selT2": _selT2_np(),
        "ones": np.ones((128, 1), dtype=np.float32),
    }
    in_maps = [dict(shared, x=x[i * BPC:(i + 1) * BPC]) for i in range(NCORES)]
    res = run_bass_kernel_spmd(nc, in_maps, list(range(NCORES)), trace=trace, **kw)
    out = np.concatenate([res.results[i]["y"] for i in range(NCORES)], axis=0)
    return out.reshape(B, C, H, W), res


def kernel(**inputs) -> np.ndarray:
    out, _ = run(inputs)
    return out



# revision 29
# speedup vs baseline: 1.1109x; 1.0186x over previous
"""AttentionBlock (GroupNorm + 4-head self-attention + proj + residual) on 8 trn2 cores.

Input  x: (16, 512, 32, 32) fp32.  Data-parallel: 2 images per NeuronCore.

Per-core dataflow (per image, C=512, N=H*W=1024, nh=4, hd=128):
  x (c,n)  --GN stats (DVE reduce + ACT square-accum + tiny PE selector mms)-->
  xn = x*a + b  (one DVE tensor_scalar per 128-chan tile)
  q,k: psum[d,n] = wqkvT[c,d].T @ xn[c,n]      (d on partitions, head-major)
  vT:  psum[n,dv] = xn[c,n].T @ wvT[c,dv]      (v born transposed; no PE transposes)
  per head h:
    scT[j,i] = k_h[c,j].T @ q_h[c,i]           (j on partitions)
    expT = exp(scale*scT)                       (ACT, PSUM->SBUF)
    S[i] += ones.T @ expT                       (PE, accumulated over j-tiles)
    av[c,i] += vT_h[j,c].T @ expT[j,i]          (PE, accumulated over j-tiles)
    r = 1/S  (DVE) -> DRAM -> DMA-broadcast to 128 partitions
    outn[c,i] = av * r                          (DVE, psum x sbuf)
  proj: pp[d,n] = wpT[c,d].T @ outn[c,n]; y = (pp + b_proj) + x   (fused DVE)

All big matmuls run in float32r (1 cycle/row on the PE vs 4 for float32).
"""

import numpy as np

import concourse.bass as bass
import concourse.bacc as bacc
import concourse.tile as tile
from concourse import mybir
from concourse.bass_utils import run_bass_kernel_spmd

F32 = mybir.dt.float32
F32R = mybir.dt.float32r
BF16 = mybir.dt.bfloat16
ATTN_BF16 = False     # q/k/vT/expT in bf16 (scores still accumulate in fp32 PSUM)
AF = mybir.ActivationFunctionType
ALU = mybir.AluOpType
AX = mybir.AxisListType

B, C, H, W = 16, 512, 32, 32
N = H * W                 # 1024
NH, HD = 4, 128
G, GS = 8, 64             # groups, channels per group
NCORES = 8
BPC = B // NCORES         # images per core
CT = C // 128             # 4 channel tiles
NT = N // 128             # 8 spatial tiles
EPS = 1e-5
SCALE = float(HD) ** -0.5
INV_GROUP = 1.0 / (GS * N)   # 1/65536


def r32(ap):
    return ap.bitcast(F32R)


def _emit(ctx, tc, aps):
    nc = tc.nc
    x_d, wqkvT_d, wpT_d, gamma_d, beta_d, bproj_d, y_d = aps[:7]

    consts = ctx.enter_context(tc.tile_pool(name="consts", bufs=1))
    xp = ctx.enter_context(tc.tile_pool(name="xp", bufs=2))
    work = ctx.enter_context(tc.tile_pool(name="work", bufs=1))
    small = ctx.enter_context(tc.tile_pool(name="small", bufs=4))
    expp = ctx.enter_context(tc.tile_pool(name="expp", bufs=3))
    rbcp = ctx.enter_context(tc.tile_pool(name="rbcp", bufs=2))
    dramp = ctx.enter_context(tc.tile_pool(name="dramp", bufs=4, space="DRAM"))
    ps_sc = ctx.enter_context(tc.tile_pool(name="ps_sc", bufs=3, space="PSUM"))
    ps_av = ctx.enter_context(tc.tile_pool(name="ps_av", bufs=3, space="PSUM"))
    ps_s = ctx.enter_context(tc.tile_pool(name="ps_s", bufs=2, space="PSUM"))

    # ---- tiny constants first (sel gates the GN matmuls), then image 0's x,
    # then the big weight DMAs (queues are FIFO: a small transfer queued after
    # a 3MB one waits for all of it)
    eps2 = consts.tile([2, 1], F32)
    nc.vector.memset(eps2, EPS)
    sel = consts.tile([128, 2], F32)
    nc.sync.dma_start(out=sel, in_=aps[7])
    selT2 = consts.tile([2, 128], F32)
    nc.sync.dma_start(out=selT2, in_=aps[8])
    gamma = consts.tile([128, CT], F32)
    nc.sync.dma_start(out=gamma, in_=gamma_d.rearrange("(t p) -> p t", p=128))
    beta = consts.tile([128, CT], F32)
    nc.sync.dma_start(out=beta, in_=beta_d.rearrange("(t p) -> p t", p=128))
    if ATTN_BF16:
        ones = consts.tile([128, 1], BF16)
        nc.vector.memset(ones, 1.0)
    else:
        ones = consts.tile([128, 1], F32R)
        nc.sync.dma_start(out=ones, in_=r32(aps[9]))

    x_tiles = []
    x0 = xp.tile([128, CT, N], F32, tag="x", name="x0")
    x0_r = x_d[0].rearrange("(t p) n -> p t n", p=128)
    for ct in range(CT):
        nc.sync.dma_start(out=x0[:, ct, :], in_=x0_r[:, ct, :])
    x_tiles.append(x0)

    wqkvT = consts.tile([128, CT, 3 * C], F32R)
    wq_r = r32(wqkvT_d.rearrange("(t p) d -> p t d", p=128))
    for ct in range(CT):
        nc.sync.dma_start(out=wqkvT[:, ct, :], in_=wq_r[:, ct, :])
    x1 = xp.tile([128, CT, N], F32, tag="x", name="x1")
    x1_r = x_d[1].rearrange("(t p) n -> p t n", p=128)
    for ct in range(CT):
        nc.sync.dma_start(out=x1[:, ct, :], in_=x1_r[:, ct, :])
    x_tiles.append(x1)
    wpT = consts.tile([128, CT, C], F32R)
    nc.sync.dma_start(out=wpT, in_=r32(wpT_d.rearrange("(t p) d -> p t d", p=128)))
    bproj = consts.tile([128, CT], F32)
    nc.sync.dma_start(out=bproj, in_=bproj_d.rearrange("(t p) -> p t", p=128))

    xn_l, q_l, k_l, vT_l, outn_l = {}, {}, {}, {}, {}

    def phase_gn(b):
        x_t = x_tiles[b]
        # ---- group norm --------------------------------------------------
        xn_t = work.tile([128, CT, N], F32R, tag="xn", name=f"xn{b}")
        ss = small.tile([128, CT, 2], F32, tag="ss")
        for ct in range(CT):
            nc.vector.reduce_sum(out=ss[:, ct, 0:1], in_=x_t[:, ct, :], axis=AX.X)
            # squares go to xn_t as scratch (overwritten below)
            nc.scalar.activation(out=xn_t[:, ct, :], in_=x_t[:, ct, :],
                                 func=AF.Square, accum_out=ss[:, ct, 1:2])
        gn_ps = ps_sc.tile([2, CT, 2], F32, tag="sc")
        for ct in range(CT):
            nc.tensor.matmul(gn_ps[:, ct, :], lhsT=sel, rhs=ss[:, ct, :],
                             start=(ct == 0), stop=(ct == CT - 1))
        msq = small.tile([2, CT, 1], F32, tag="msq")
        nc.scalar.activation(out=msq, in_=gn_ps[:, :, 0:1], func=AF.Square)
        var = small.tile([2, CT, 1], F32, tag="var")
        nc.vector.tensor_sub(out=var, in0=gn_ps[:, :, 1:2], in1=msq)
        std = small.tile([2, CT, 1], F32, tag="std")
        nc.scalar.activation(out=std, in_=var, func=AF.Sqrt, bias=eps2)
        mr = small.tile([2, CT, 2], F32, tag="mr")      # [mean, rstd]
        nc.vector.reciprocal(out=mr[:, :, 1:2], in_=std)
        nc.vector.tensor_copy(out=mr[:, :, 0:1], in_=gn_ps[:, :, 0:1])
        for ct in range(CT):
            bc = ps_sc.tile([128, 2], F32, tag="sc")
            nc.tensor.matmul(bc, lhsT=selT2, rhs=mr[:, ct, :],
                             start=True, stop=True)     # [mean_c, rstd_c]
            a_sc = small.tile([128, 1], F32, tag="a_sc")
            nc.vector.tensor_mul(out=a_sc, in0=bc[:, 1:2], in1=gamma[:, ct:ct + 1])
            nb_sc = small.tile([128, 1], F32, tag="nb_sc")  # mean*a - beta
            nc.vector.scalar_tensor_tensor(out=nb_sc, in0=bc[:, 0:1], scalar=a_sc,
                                           in1=beta[:, ct:ct + 1],
                                           op0=ALU.mult, op1=ALU.subtract)
            nc.vector.tensor_scalar(out=xn_t[:, ct, :], in0=x_t[:, ct, :],
                                    scalar1=a_sc, scalar2=nb_sc,
                                    op0=ALU.mult, op1=ALU.subtract)

        xn_l[b] = xn_t

    def phase_qkv(b):
        xn_t = xn_l[b]
        # ---- q, k --------------------------------------------------------
        adt = BF16 if ATTN_BF16 else F32R
        q_t = work.tile([128, NH, N], adt, tag="q", name=f"q{b}", bufs=1)
        k_t = work.tile([128, NH, N], adt, tag="k", name=f"k{b}", bufs=1)
        for which, dst in ((0, q_t), (1, k_t)):
            for h in range(NH):
                dlo = which * C + h * 128
                for nh_ in range(2):
                    nsl = slice(nh_ * 512, (nh_ + 1) * 512)
                    qk_ps = ps_sc.tile([128, 512], F32, tag="sc")
                    for ct in range(CT):
                        nc.tensor.matmul(
                            qk_ps, lhsT=wqkvT[:, ct, dlo:dlo + 128],
                            rhs=xn_t[:, ct, nsl],
                            start=(ct == 0), stop=(ct == CT - 1))
                    if which == 0:
                        nc.scalar.copy(out=dst[:, h, nsl], in_=qk_ps)
                    else:
                        nc.vector.tensor_copy(out=dst[:, h, nsl], in_=qk_ps)

        q_l[b], k_l[b] = q_t, k_t

    def phase_vt(b):
        xn_t = xn_l[b]
        adt = BF16 if ATTN_BF16 else F32R
        # ---- vT ----------------------------------------------------------
        vT_t = work.tile([128, NT, C], adt, tag="vT", name=f"vT{b}", bufs=2)
        for nt in range(NT):
            vt_ps = ps_sc.tile([128, C], F32, tag="sc")
            for ct in range(CT):
                nc.tensor.matmul(vt_ps,
                                 lhsT=xn_t[:, ct, nt * 128:(nt + 1) * 128],
                                 rhs=wqkvT[:, ct, 2 * C:3 * C],
                                 start=(ct == 0), stop=(ct == CT - 1))
            nc.vector.tensor_copy(out=vT_t[:, nt, :], in_=vt_ps)
        vT_l[b] = vT_t

    def phase_b(b):
        q_t, k_t, vT_t = q_l[b], k_l[b], vT_l[b]
        adt = BF16 if ATTN_BF16 else F32R
        outn_t = work.tile([128, NH, N], F32R, tag="outn", name=f"outn{b}", bufs=1)
        for h in range(NH):
            for ih in range(2):
                isl = slice(ih * 512, (ih + 1) * 512)
                s_ps = ps_s.tile([1, 512], F32, tag="s")
                av = ps_av.tile([128, 512], F32, tag="av")
                for jt in range(NT):
                    sc = ps_sc.tile([128, 512], F32, tag="sc")
                    nc.tensor.matmul(sc,
                                     lhsT=k_t[:, h, jt * 128:(jt + 1) * 128],
                                     rhs=q_t[:, h, isl], start=True, stop=True)
                    expT = expp.tile([128, 512], adt, tag="expT")
                    nc.scalar.activation(out=expT, in_=sc, func=AF.Exp, scale=SCALE)
                    nc.tensor.matmul(s_ps, lhsT=ones, rhs=expT,
                                     start=(jt == 0), stop=(jt == NT - 1))
                    nc.tensor.matmul(av,
                                     lhsT=vT_t[:, jt, h * 128:(h + 1) * 128],
                                     rhs=expT,
                                     start=(jt == 0), stop=(jt == NT - 1))
                # 1/S on a (128,4) layout (a (1,512) reciprocal is ~3us: the
                # iterative divide runs on one lane); round-trip through DRAM
                s_sb = small.tile([1, 512], F32, tag="s_sb")
                nc.vector.tensor_copy(out=s_sb, in_=s_ps)
                s128 = small.tile([128, 4], F32, tag="s128")
                nc.sync.dma_start(
                    out=s128,
                    in_=bass.AP(tensor=s_sb.tensor, offset=s_sb.offset,
                                ap=[list(s_sb.ap[0]), [4, 128], [1, 4]]))
                r128 = small.tile([128, 4], F32, tag="r128")
                nc.vector.reciprocal(out=r128, in_=s128)
                r_dram = dramp.tile([512], F32, tag="r_dram")
                nc.sync.dma_start(out=r_dram.rearrange("(p f) -> p f", p=128),
                                  in_=r128)
                r_bc = rbcp.tile([128, 512], F32, tag="r_bc")
                nc.sync.dma_start(
                    out=r_bc,
                    in_=bass.AP(tensor=r_dram.tensor, offset=r_dram.offset,
                                ap=[[0, 128]] + list(r_dram.ap)))
                nc.vector.tensor_mul(out=outn_t[:, h, isl], in0=av, in1=r_bc)
        outn_l[b] = outn_t

    def phase_c(b):
        x_t, outn_t = x_tiles[b], outn_l[b]
        y_t = work.tile([128, CT, N], F32, tag="y", name=f"y{b}")
        for dt in range(CT):
            for nh_ in range(2):
                nsl = slice(nh_ * 512, (nh_ + 1) * 512)
                pp = ps_sc.tile([128, 512], F32, tag="sc")
                for ct in range(CT):
                    nc.tensor.matmul(
                        pp, lhsT=wpT[:, ct, dt * 128:(dt + 1) * 128],
                        rhs=outn_t[:, ct, nsl],
                        start=(ct == 0), stop=(ct == CT - 1))
                nc.vector.scalar_tensor_tensor(out=y_t[:, dt, nsl], in0=pp,
                                               scalar=bproj[:, dt:dt + 1],
                                               in1=x_t[:, dt, nsl],
                                               op0=ALU.add, op1=ALU.add)
            nc.sync.dma_start(
                out=y_d[b].rearrange("(t p) n -> p t n", p=128)[:, dt, :],
                in_=y_t[:, dt, :])

    phase_gn(0)
    phase_qkv(0)
    phase_vt(0)
    phase_b(0)
    phase_gn(1)
    phase_vt(1)
    phase_c(0)
    phase_qkv(1)
    phase_b(1)
    phase_c(1)


def _sel_np():
    s = np.zeros((128, 2), dtype=np.float32)
    s[0:GS, 0] = INV_GROUP
    s[GS:128, 1] = INV_GROUP
    return s


def _selT2_np():
    s = np.zeros((2, 128), dtype=np.float32)
    s[0, 0:GS] = 1.0
    s[1, GS:128] = 1.0
    return s


_CACHE = {}


def _build():
    if "nc" in _CACHE:
        return _CACHE["nc"]
    nc = bacc.Bacc("TRN2", target_bir_lowering=False, debug=False)
    x_d = nc.dram_tensor("x", [BPC, C, N], F32, kind="ExternalInput").ap()
    wqkvT_d = nc.dram_tensor("wqkvT", [C, 3 * C], F32, kind="ExternalInput").ap()
    wpT_d = nc.dram_tensor("wpT", [C, C], F32, kind="ExternalInput").ap()
    gamma_d = nc.dram_tensor("gamma", [C], F32, kind="ExternalInput").ap()
    beta_d = nc.dram_tensor("beta", [C], F32, kind="ExternalInput").ap()
    bproj_d = nc.dram_tensor("bproj", [C], F32, kind="ExternalInput").ap()
    sel_d = nc.dram_tensor("sel", [128, 2], F32, kind="ExternalInput").ap()
    selT2_d = nc.dram_tensor("selT2", [2, 128], F32, kind="ExternalInput").ap()
    ones_d = nc.dram_tensor("ones", [128, 1], F32, kind="ExternalInput").ap()
    y_d = nc.dram_tensor("y", [BPC, C, N], F32, kind="ExternalOutput").ap()
    from contextlib import ExitStack
    with tile.TileContext(nc) as tc, ExitStack() as ctx:
        _emit(ctx, tc, (x_d, wqkvT_d, wpT_d, gamma_d, beta_d, bproj_d, y_d,
                        sel_d, selT2_d, ones_d))
    nc.compile()
    _CACHE["nc"] = nc
    return nc


def run(inputs: dict, trace: bool = False, **kw):
    nc = _build()
    x = np.ascontiguousarray(inputs["x"].reshape(B, C, N), dtype=np.float32)
    shared = {
        "wqkvT": np.ascontiguousarray(inputs["w_qkv"].T, dtype=np.float32),
        "wpT": np.ascontiguousarray(inputs["w_proj"].T, dtype=np.float32),
        "gamma": np.ascontiguousarray(inputs["gn_gamma"], dtype=np.float32),
        "beta": np.ascontiguousarray(inputs["gn_beta"], dtype=np.float32),
        "bproj": np.ascontiguousarray(inputs["b_proj"], dtype=np.float32),
        "sel": _sel_np(),
        "selT2": _selT2_np(),
        "ones": np.ones((128, 1), dtype=np.float32),
    }
    in_maps = [dict(shared, x=x[i * BPC:(i + 1) * BPC]) for i in range(NCORES)]
    res = run_bass_kernel_spmd(nc, in_maps, list(range(NCORES)), trace=trace, **kw)
    out = np.concatenate([res.results[i]["y"] for i in range(NCORES)], axis=0)
    return out.reshape(B, C, H, W), res


def kernel(**inputs) -> np.ndarray:
    out, _ = run(inputs)
    return out

